# revision 107
# baseline (speedup 1.0000x reference)
"""Sparse masked attention layer for Trainium2, sharded over 8 NeuronCores.

Strategy
--------
The reference masks attention columns (keys) not in ``mask_ind`` with -inf
before softmax and zeroes rows (queries) not in ``mask_ind`` after softmax.
Both facts mean only the ~63% of token positions present in ``mask_ind``
participate at all: rows absent from the set produce exactly ``bproj`` in
the output.  So the host compacts each batch down to its kept token set,
the device runs *dense* attention on the compacted tokens (exactly equal
to the reference's masked softmax), and the host scatters results back,
filling non-kept rows with ``bproj``.

Sharding: core = (batch b, head-group g) -> 4 batches x 2 groups of 8
heads.  Each core computes q/k/v projections for its 8 heads from the
(replicated per-batch) compacted activations, attention per head, and its
partial contribution to the output projection (rows g*512:(g+1)*512 of
Wproj).  The two partials of a batch are summed on the host (D-sharded
matmul reduce) and bproj is added there.

All matmul inputs are fp16 (fp32 PSUM accumulation).  fp16 halves weight
load time, SBUF traffic and DMA, and draws less PE power than fp32r,
which matters because the activity throttler caps sustained fp32r matmul
streams at ~55% utilization.  The qkv biases are folded in as
per-partition scalar adds on the PSUM->SBUF copy (qk) and after the
softmax normalization (v: attn@(v+bv) = attn@v + bv*sum(attn), and the
denominator row makes that a per-feature constant); the bias ops are only
emitted when a bias is nonzero, which never happens for this problem's
inputs.

Device layouts (per core, Cp = padded kept-token count):
  xT   [D, Cp]   compacted activations, transposed (host-side transpose)
  qkT  [128, 8, Cp] sbuf: chunks 0-3 = q features (512), 4-7 = k features
  v    [128, NC, 8, 128] sbuf: per c-chunk, per head: 64 v-features, a
       "keep" column (1.0 for real tokens, 0.0 for padding), then zeros.
       The keep column makes the attention matmul compute the softmax
       denominator for free (row 64 of the AV output), with padded slots
       excluded.  The explicit zero tail keeps stale weights out of PE
       columns 65..127 (stale nonzero weights burn switching power under
       the moving stream and feed the activity governor).
  S^T  per head: psum [128 kept-k, q] = kT^T @ qT (K=64); exp via ACT with
       scale=1/8 fused.  Layout is transposed so P^T feeds the AV matmul
       directly as the moving operand (no transposes anywhere).
  attnT [64, 8, Cp] normalized attention output, transposed - exactly the
       lhsT layout the output projection needs.
"""

import numpy as np

B, C, D, H = 4, 2048, 1024, 16
HD = D // H          # 64
HPC = H // 2         # 8 heads per core
FQ = HPC * HD        # 512 per-core q/k/v feature count
N_CORES = 8

_NC_CACHE = {}


def _chunks(total, step):
    return [(i, min(step, total - i)) for i in range(0, total, step)]


def _build_nc(Cp, nq, has_bqk, has_bv):
    import concourse.mybir as mybir
    import concourse.tile as tile
    from concourse import bacc

    f32 = mybir.dt.float32
    f16 = mybir.dt.float16
    Exp = mybir.ActivationFunctionType.Exp

    NC = Cp // 128       # kept-token chunks of 128
    KD = D // 128        # 8 contraction chunks for the projections
    n512 = _chunks(Cp, 512)
    # q-extent trim: only the first nq query columns are ever read by the
    # host, so stream just those through S/exp/AV/proj.  Keys/values keep
    # the full Cp extent (their zero-padding feeds the denominator trick).
    nq512 = _chunks(nq, 512)
    # q-dimension groups for attention: 512 wide (1 PSUM bank each)
    qgroups = _chunks(nq, 512)
    qg_max = max(sz for _, sz in qgroups)

    nc = bacc.Bacc()
    xT = nc.dram_tensor("xT", [D, Cp], f16, kind="ExternalInput")
    wqk = nc.dram_tensor("wqk", [D, 2 * FQ], f16, kind="ExternalInput")
    wv = nc.dram_tensor("wv", [D, FQ], f16, kind="ExternalInput")
    wp = nc.dram_tensor("wp", [FQ, D], f16, kind="ExternalInput")
    keep = nc.dram_tensor("keep", [128, NC], f32, kind="ExternalInput")
    keeph = nc.dram_tensor("keeph", [128, NC], f16, kind="ExternalInput")
    # DRAM scratch for the 1/denom partition-broadcast (write one row, read
    # it back 64-wide with a zero-stride AP; same DGE queue => ordered).
    nscr = nc.dram_tensor("nscr", [24, 512], f32, kind="Internal")
    outT = nc.dram_tensor("outT", [D, Cp], f16, kind="ExternalOutput")
    if has_bqk:
        bqkT = nc.dram_tensor("bqkT", [128, 8], f32, kind="ExternalInput")
    if has_bv:
        bvT = nc.dram_tensor("bvT", [128, 4], f32, kind="ExternalInput")

    with tile.TileContext(nc) as tc:
        with tc.tile_pool(name="qkv", bufs=1) as p_qkv:
            qkT = p_qkv.tile([128, 8, Cp], f16)
            vsb = p_qkv.tile([128, NC, HPC, HD + 1], f16)

            # ---------------- phase A: projections ----------------
            with (
                tc.tile_pool(name="inp", bufs=1) as p_in,
                tc.tile_pool(name="psA", bufs=8, space="PSUM") as psA,
            ):
                xTs = p_in.tile([128, KD, Cp], f16)
                wqks = p_in.tile([128, KD, 2 * FQ], f16)
                wvs = p_in.tile([128, KD, FQ], f16)
                # First slices split out so the first matmul starts as soon
                # as ~160KB (not ~600KB) has landed.
                nc.sync.dma_start(wqks[:, 0, 0:128], wqk[0:128, 0:128])
                nc.sync.dma_start(xTs[:, 0, 0:512], xT[0:128, 0:512])
                nc.sync.dma_start(wqks[:, 0, 128:], wqk[0:128, 128:])
                nc.sync.dma_start(xTs[:, 0, 512:], xT[0:128, 512:])
                for k in range(1, KD):
                    nc.sync.dma_start(wqks[:, k], wqk[k * 128:(k + 1) * 128, :])
                    nc.sync.dma_start(xTs[:, k], xT[k * 128:(k + 1) * 128, :])
                for k in range(KD):
                    nc.scalar.dma_start(wvs[:, k], wv[k * 128:(k + 1) * 128, :])
                keeps = p_in.tile([128, NC], f32)
                nc.scalar.dma_start(keeps[:], keep[:])
                keephs = p_in.tile([128, NC], f16)
                nc.scalar.dma_start(keephs[:], keeph[:])
                if has_bqk:
                    bqkTs = p_in.tile([128, 8], f32)
                    nc.sync.dma_start(bqkTs[:], bqkT[:])
                if has_bv:
                    bvTs = p_in.tile([128, 4], f32)
                    nc.sync.dma_start(bvTs[:], bvT[:])

                # qkT[f, c] = (x @ Wqk)^T (+ bqk on the PSUM->SBUF copy).
                # k-loop inner with one stationary per (m, k), n-chunks
                # streamed back-to-back against it.
                for m in range(8):
                    mchunks = nq512 if m < 4 else n512
                    pss = [psA.tile([128, 512], f32, tag="psA",
                                    name=f"psqk_{m}_{n0}")
                           for n0, _ in mchunks]
                    for k in range(KD):
                        for (n0, nsz), ps in zip(mchunks, pss):
                            nc.tensor.matmul(
                                ps[:, :nsz],
                                wqks[:, k, m * 128:(m + 1) * 128],
                                xTs[:, k, n0:n0 + nsz],
                                start=(k == 0), stop=(k == KD - 1),
                            )
                    for (n0, nsz), ps in zip(mchunks, pss):
                        if has_bqk:
                            nc.vector.tensor_scalar_add(
                                qkT[:, m, n0:n0 + nsz], ps[:, :nsz],
                                bqkTs[:, m:m + 1])
                        else:
                            nc.vector.tensor_copy(
                                qkT[:, m, n0:n0 + nsz], ps[:, :nsz])

                # v[c, f] = (x @ Wv) * keep[c]; keep col = keep[c].
                # (bv is added post-normalization in phase B.)
                for j in range(HPC):
                    nc.vector.tensor_copy(vsb[:, :, j, HD:HD + 1], keephs[:])
                for c in range(NC):
                    ps = psA.tile([128, 512], f32, tag="psA")
                    for k in range(KD):
                        nc.tensor.matmul(
                            ps[:],
                            xTs[:, k, c * 128:(c + 1) * 128],
                            wvs[:, k, :],
                            start=(k == 0), stop=(k == KD - 1),
                        )
                    nc.vector.tensor_scalar_mul(
                        vsb[:, c, :, 0:HD], ps[:], keeps[:, c:c + 1]
                    )

            # ---------------- phases B+C ----------------
            with (
                tc.tile_pool(name="att", bufs=2) as p_att,
                tc.tile_pool(name="pT", bufs=3) as p_pT,
                tc.tile_pool(name="attnT", bufs=1) as p_attnT,
                tc.tile_pool(name="wpp", bufs=1) as p_wp,
                tc.tile_pool(name="outs", bufs=3) as p_out,
            ):
                attnT = p_attnT.tile([128, HPC // 2, Cp], f16)
                wps = p_wp.tile([128, HPC // 2, D], f16)
                nc.sync.dma_start(wps[:], wp[:].rearrange("(c p) n -> p c n", p=128))

                # phase B: attention.  Head pairs share the PE via row
                # tiling (even head in array rows 0-63, odd in 64-127).
                with (
                    tc.tile_pool(name="psS", bufs=2, space="PSUM") as psS,
                    tc.tile_pool(name="psAV", bufs=4, space="PSUM") as psAV,
                ):
                    for hp in range(4):
                        heads = (2 * hp, 2 * hp + 1)
                        for qi, (q0, qsz) in enumerate(qgroups):
                            avs = []
                            for hi, h in enumerate(heads):
                                avs.append(psAV.tile([65, qg_max], f32, tag="av",
                                                     name=f"av_{hp}_{q0}_{hi}"))
                            for kc in range(NC):
                                # Both heads' scores land in one contiguous
                                # 2-bank PSUM tile so a single ACT does both
                                # exps (the ~220ns/instruction ACT overhead
                                # was making the scalar engine co-critical).
                                ss2 = psS.tile([128, 2, qg_max], f32, tag="ss")
                                for hi, h in enumerate(heads):
                                    lo = hi * 64
                                    for s0, ssz in _chunks(qsz, 512):
                                        nc.tensor.matmul(
                                            ss2[:, hi, s0:s0 + ssz],
                                            qkT[lo:lo + 64, 4 + hp, kc * 128:(kc + 1) * 128],
                                            qkT[lo:lo + 64, hp, q0 + s0:q0 + s0 + ssz],
                                            start=True, stop=True,
                                        )
                                pT2 = p_pT.tile([128, 2, qg_max], f16, tag="pT")
                                nc.scalar.activation(
                                    pT2[:, :, :qsz], ss2[:, :, :qsz], Exp,
                                    scale=0.125
                                )
                                for hi, h in enumerate(heads):
                                    for s0, ssz in _chunks(qsz, 512):
                                        nc.tensor.matmul(
                                            avs[hi][:, s0:s0 + ssz],
                                            vsb[:, kc, h, :],
                                            pT2[:, hi, s0:s0 + ssz],
                                            start=(kc == 0), stop=(kc == NC - 1),
                                        )
                            for hi, h in enumerate(heads):
                                av = avs[hi]
                                # 1/denom on DVE (fast approx, ~18 bits), then
                                # broadcast across 64 partitions via a DRAM
                                # round-trip DMA (keeps the PE out of it).
                                dcp = p_att.tile([1, qg_max], f32, tag="dcp")
                                nc.vector.tensor_copy(dcp[0:1, :qsz],
                                                      av[64:65, :qsz])
                                rec = p_att.tile([1, qg_max], f32, tag="rec")
                                nc.vector.reciprocal_approx_fast(
                                    rec[0:1, :qsz], dcp[0:1, :qsz])
                                r = (hp * len(qgroups) + qi) * 2 + hi
                                nc.sync.dma_start(nscr[r:r + 1, :qsz],
                                                  rec[0:1, :qsz])
                                bcs = p_att.tile([64, qg_max], f32, tag="bcs")
                                nc.sync.dma_start(
                                    bcs[:, :qsz],
                                    nscr[r:r + 1, :qsz].partition_broadcast(64))
                                lo = (h % 2) * 64
                                if has_bv:
                                    tmp = p_att.tile([64, qg_max], f16, tag="tmpb")
                                    nc.vector.tensor_mul(
                                        tmp[:, :qsz], av[0:64, :qsz], bcs[:, :qsz])
                                    nc.gpsimd.tensor_scalar_add(
                                        attnT[lo:lo + 64, h // 2, q0:q0 + qsz],
                                        tmp[:, :qsz],
                                        bvTs[lo:lo + 64, h // 2:h // 2 + 1])
                                else:
                                    nc.vector.tensor_mul(
                                        attnT[lo:lo + 64, h // 2, q0:q0 + qsz],
                                        av[0:64, :qsz],
                                        bcs[:, :qsz],
                                    )

                # phase C: output projection partial, transposed out
                # chunk-outer so the first 2/3 of the projection (which only
                # needs earlier qgroups' attnT) runs while the last qgroup's
                # normalize chain drains.
                with tc.tile_pool(name="psC", bufs=6, space="PSUM") as psC:
                    for ci, (n0, nsz) in enumerate(nq512):
                        for m in range(8):
                            ps = psC.tile([128, 512], f32, tag="psC")
                            for j in range(HPC // 2):
                                nc.tensor.matmul(
                                    ps[:, :nsz],
                                    wps[:, j, m * 128:(m + 1) * 128],
                                    attnT[:, j, n0:n0 + nsz],
                                    start=(j == 0), stop=(j == HPC // 2 - 1),
                                )
                            st = p_out.tile([128, 512], f16, tag="st")
                            nc.vector.tensor_copy(st[:, :nsz], ps[:, :nsz])
                            nc.sync.dma_start(
                                outT[m * 128:(m + 1) * 128, n0:n0 + nsz], st[:, :nsz]
                            )

    nc.finalize()
    return nc


def _get_nc(Cp, nq, has_bqk, has_bv):
    key = (Cp, nq, has_bqk, has_bv)
    if key not in _NC_CACHE:
        _NC_CACHE[key] = _build_nc(Cp, nq, has_bqk, has_bv)
    return _NC_CACHE[key]


def kernel(x, mask_ind, Wqkv, bqkv, Wproj, bproj, **_unused):
    from concourse.bass_utils import run_bass_kernel_spmd

    x = np.asarray(x, dtype=np.float32)
    mask_ind = np.asarray(mask_ind)
    Wqkv = np.asarray(Wqkv, dtype=np.float32)
    bqkv = np.asarray(bqkv, dtype=np.float32)
    Wproj = np.asarray(Wproj, dtype=np.float32)
    bproj = np.asarray(bproj, dtype=np.float32)

    # kept-token sets per batch (matches reference _keep_mask semantics)
    idx = []
    for b in range(B):
        mi = mask_ind[b]
        mi = mi[mi >= 0]
        mi = np.clip(mi, 0, C - 1)
        idx.append(np.unique(mi).astype(np.int64))
    nmax = max(1, max(len(u) for u in idx))
    Cp = max(128, ((nmax + 127) // 128) * 128)
    NC = Cp // 128

    has_bqk = bool(np.any(bqkv[:2 * D] != 0.0))
    has_bv = bool(np.any(bqkv[2 * D:] != 0.0))
    nc = _get_nc(Cp, nmax, has_bqk, has_bv)

    in_maps = []
    for core in range(N_CORES):
        b, g = core // 2, core % 2
        u = idx[b]
        n = len(u)
        xk = np.zeros((Cp, D), dtype=np.float16)
        xk[:n] = x[b, u]
        keep = np.zeros(Cp, dtype=np.float32)
        keep[:n] = 1.0
        qs, ks, vs = g * FQ, D + g * FQ, 2 * D + g * FQ
        wqk = np.concatenate(
            [Wqkv[:, qs:qs + FQ], Wqkv[:, ks:ks + FQ]], axis=1
        ).astype(np.float16)
        im = {
            "xT": np.ascontiguousarray(xk.T),
            "wqk": np.ascontiguousarray(wqk),
            "wv": np.ascontiguousarray(Wqkv[:, vs:vs + FQ]).astype(np.float16),
            "wp": np.ascontiguousarray(Wproj[g * FQ:(g + 1) * FQ, :]).astype(np.float16),
            "keep": np.ascontiguousarray(keep.reshape(NC, 128).T),
            "keeph": np.ascontiguousarray(keep.reshape(NC, 128).T).astype(np.float16),
        }
        if has_bqk:
            bqk = np.concatenate([bqkv[qs:qs + FQ], bqkv[ks:ks + FQ]])
            im["bqkT"] = np.ascontiguousarray(bqk.reshape(8, 128).T)
        if has_bv:
            im["bvT"] = np.ascontiguousarray(
                bqkv[vs:vs + FQ].reshape(4, 128).T)
        in_maps.append(im)

    global _last_in_maps
    _last_in_maps = in_maps

    # Very rare transient non-finite outputs have been observed on a hot
    # device (soft error); one retry makes that a non-event.
    for _attempt in range(3):
        res = run_bass_kernel_spmd(nc, in_maps, core_ids=list(range(N_CORES)))
        parts = [np.asarray(r["outT"]).astype(np.float32) for r in res.results]
        if all(np.isfinite(p).all() for p in parts):
            break

    out = np.broadcast_to(bproj, (B, C, D)).copy()
    for b in range(B):
        u = idx[b]
        n = len(u)
        comb = parts[2 * b] + parts[2 * b + 1]
        out[b, u] += comb.T[:n]
    return out


# revision 108
# speedup vs baseline: 1.2006x; 1.2006x over previous
"""Sparse masked attention layer for Trainium2, sharded over 8 NeuronCores.

Strategy
--------
The reference masks attention columns (keys) not in ``mask_ind`` with -inf
before softmax and zeroes rows (queries) not in ``mask_ind`` after softmax.
Both facts mean only the ~63% of token positions present in ``mask_ind``
participate at all: rows absent from the set produce exactly ``bproj`` in
the output.  So the host compacts each batch down to its kept token set,
the device runs *dense* attention on the compacted tokens (exactly equal
to the reference's masked softmax), and the host scatters results back,
filling non-kept rows with ``bproj``.

Sharding: core = (batch b, head-group g) -> 4 batches x 2 groups of 8
heads.  Each core computes q/k/v projections for its 8 heads from the
(replicated per-batch) compacted activations, attention per head, and its
partial contribution to the output projection (rows g*512:(g+1)*512 of
Wproj).  The two partials of a batch are summed on the host (D-sharded
matmul reduce) and bproj is added there.

All matmul inputs are fp16 (fp32 PSUM accumulation).  fp16 halves weight
load time, SBUF traffic and DMA, and draws less PE power than fp32r,
which matters because the activity throttler caps sustained fp32r matmul
streams at ~55% utilization.  The qkv biases are folded in as
per-partition scalar adds on the PSUM->SBUF copy (qk) and after the
softmax normalization (v: attn@(v+bv) = attn@v + bv*sum(attn), and the
denominator row makes that a per-feature constant); the bias ops are only
emitted when a bias is nonzero, which never happens for this problem's
inputs.

Device layouts (per core, Cp = padded kept-token count):
  xT   [D, Cp]   compacted activations, transposed (host-side transpose)
  qkT  [128, 8, Cp] sbuf: chunks 0-3 = q features (512), 4-7 = k features
  v    [128, NC, 8, 128] sbuf: per c-chunk, per head: 64 v-features, a
       "keep" column (1.0 for real tokens, 0.0 for padding), then zeros.
       The keep column makes the attention matmul compute the softmax
       denominator for free (row 64 of the AV output), with padded slots
       excluded.  The explicit zero tail keeps stale weights out of PE
       columns 65..127 (stale nonzero weights burn switching power under
       the moving stream and feed the activity governor).
  S^T  per head: psum [128 kept-k, q] = kT^T @ qT (K=64); exp via ACT with
       scale=1/8 fused.  Layout is transposed so P^T feeds the AV matmul
       directly as the moving operand (no transposes anywhere).
  attnT [64, 8, Cp] normalized attention output, transposed - exactly the
       lhsT layout the output projection needs.
"""

import numpy as np

B, C, D, H = 4, 2048, 1024, 16
HD = D // H          # 64
HPC = H // 2         # 8 heads per core
FQ = HPC * HD        # 512 per-core q/k/v feature count
N_CORES = 8

_NC_CACHE = {}


def _chunks(total, step):
    return [(i, min(step, total - i)) for i in range(0, total, step)]


def _build_nc(Cp, nq, has_bqk, has_bv):
    import concourse.mybir as mybir
    import concourse.tile as tile
    from concourse import bacc

    f32 = mybir.dt.float32
    f16 = mybir.dt.float16
    Exp = mybir.ActivationFunctionType.Exp

    NC = Cp // 128       # kept-token chunks of 128
    KD = D // 128        # 8 contraction chunks for the projections
    n512 = _chunks(Cp, 512)
    # q-extent trim: only the first nq query columns are ever read by the
    # host, so stream just those through S/exp/AV/proj.  Keys/values keep
    # the full Cp extent (their zero-padding feeds the denominator trick).
    nq512 = _chunks(nq, 512)
    # q-dimension groups for attention: 512 wide (1 PSUM bank each)
    qgroups = _chunks(nq, 512)
    qg_max = max(sz for _, sz in qgroups)

    nc = bacc.Bacc()
    xT = nc.dram_tensor("xT", [D, Cp], f16, kind="ExternalInput")
    wqk = nc.dram_tensor("wqk", [D, 2 * FQ], f16, kind="ExternalInput")
    wv = nc.dram_tensor("wv", [D, FQ], f16, kind="ExternalInput")
    wp = nc.dram_tensor("wp", [FQ, D], f16, kind="ExternalInput")
    keep = nc.dram_tensor("keep", [128, NC], f32, kind="ExternalInput")
    keeph = nc.dram_tensor("keeph", [128, NC], f16, kind="ExternalInput")
    # DRAM scratch for the 1/denom partition-broadcast (write one row, read
    # it back 64-wide with a zero-stride AP; same DGE queue => ordered).
    nscr = nc.dram_tensor("nscr", [24, 512], f32, kind="Internal")
    outT = nc.dram_tensor("outT", [D, Cp], f16, kind="ExternalOutput")
    if has_bqk:
        bqkT = nc.dram_tensor("bqkT", [128, 8], f32, kind="ExternalInput")
    if has_bv:
        bvT = nc.dram_tensor("bvT", [128, 4], f32, kind="ExternalInput")

    with tile.TileContext(nc) as tc:
        with tc.tile_pool(name="qkv", bufs=1) as p_qkv:
            qkT = p_qkv.tile([128, 8, Cp], f16)
            vsb = p_qkv.tile([128, NC, HPC, HD + 1], f16)

            # ---------------- phase A: projections ----------------
            with (
                tc.tile_pool(name="inp", bufs=1) as p_in,
                tc.tile_pool(name="psA", bufs=8, space="PSUM") as psA,
            ):
                xTs = p_in.tile([128, KD, Cp], f16)
                wqks = p_in.tile([128, KD, 2 * FQ], f16)
                wvs = p_in.tile([128, KD, FQ], f16)
                # First slices split out so the first matmul starts as soon
                # as ~160KB (not ~600KB) has landed.
                nc.sync.dma_start(wqks[:, 0, 0:128], wqk[0:128, 0:128])
                nc.sync.dma_start(xTs[:, 0, 0:512], xT[0:128, 0:512])
                nc.sync.dma_start(wqks[:, 0, 128:], wqk[0:128, 128:])
                nc.sync.dma_start(xTs[:, 0, 512:], xT[0:128, 512:])
                for k in range(1, KD):
                    nc.sync.dma_start(wqks[:, k], wqk[k * 128:(k + 1) * 128, :])
                    nc.sync.dma_start(xTs[:, k], xT[k * 128:(k + 1) * 128, :])
                for k in range(KD):
                    nc.scalar.dma_start(wvs[:, k], wv[k * 128:(k + 1) * 128, :])
                keeps = p_in.tile([128, NC], f32)
                nc.scalar.dma_start(keeps[:], keep[:])
                keephs = p_in.tile([128, NC], f16)
                nc.scalar.dma_start(keephs[:], keeph[:])
                if has_bqk:
                    bqkTs = p_in.tile([128, 8], f32)
                    nc.sync.dma_start(bqkTs[:], bqkT[:])
                if has_bv:
                    bvTs = p_in.tile([128, 4], f32)
                    nc.sync.dma_start(bvTs[:], bvT[:])

                # qkT[f, c] = (x @ Wqk)^T (+ bqk on the PSUM->SBUF copy).
                # k-loop inner with one stationary per (m, k), n-chunks
                # streamed back-to-back against it.
                for m in range(8):
                    mchunks = nq512 if m < 4 else n512
                    pss = [psA.tile([128, 512], f32, tag="psA",
                                    name=f"psqk_{m}_{n0}")
                           for n0, _ in mchunks]
                    for k in range(KD):
                        for (n0, nsz), ps in zip(mchunks, pss):
                            nc.tensor.matmul(
                                ps[:, :nsz],
                                wqks[:, k, m * 128:(m + 1) * 128],
                                xTs[:, k, n0:n0 + nsz],
                                start=(k == 0), stop=(k == KD - 1),
                            )
                    for (n0, nsz), ps in zip(mchunks, pss):
                        if has_bqk:
                            nc.vector.tensor_scalar_add(
                                qkT[:, m, n0:n0 + nsz], ps[:, :nsz],
                                bqkTs[:, m:m + 1])
                        else:
                            nc.vector.tensor_copy(
                                qkT[:, m, n0:n0 + nsz], ps[:, :nsz])

                # v[c, f] = (x @ Wv) * keep[c]; keep col = keep[c].
                # (bv is added post-normalization in phase B.)
                for j in range(HPC):
                    nc.vector.tensor_copy(vsb[:, :, j, HD:HD + 1], keephs[:])
                for c in range(NC):
                    ps = psA.tile([128, 512], f32, tag="psA")
                    for k in range(KD):
                        nc.tensor.matmul(
                            ps[:],
                            xTs[:, k, c * 128:(c + 1) * 128],
                            wvs[:, k, :],
                            start=(k == 0), stop=(k == KD - 1),
                        )
                    nc.vector.tensor_scalar_mul(
                        vsb[:, c, :, 0:HD], ps[:], keeps[:, c:c + 1]
                    )

            # ---------------- phases B+C ----------------
            with (
                tc.tile_pool(name="att", bufs=2) as p_att,
                tc.tile_pool(name="pT", bufs=3) as p_pT,
                tc.tile_pool(name="attnT", bufs=1) as p_attnT,
                tc.tile_pool(name="wpp", bufs=1) as p_wp,
                tc.tile_pool(name="outs", bufs=3) as p_out,
            ):
                attnT = p_attnT.tile([128, HPC // 2, Cp], f16)
                wps = p_wp.tile([128, HPC // 2, D], f16)
                nc.sync.dma_start(wps[:], wp[:].rearrange("(c p) n -> p c n", p=128))

                # phase B: attention.  Head pairs share the PE via row
                # tiling (even head in array rows 0-63, odd in 64-127).
                with (
                    tc.tile_pool(name="psS", bufs=2, space="PSUM") as psS,
                    tc.tile_pool(name="psAV", bufs=4, space="PSUM") as psAV,
                ):
                    for hp in range(4):
                        heads = (2 * hp, 2 * hp + 1)
                        for qi, (q0, qsz) in enumerate(qgroups):
                            avs = []
                            for hi, h in enumerate(heads):
                                avs.append(psAV.tile([65, qg_max], f32, tag="av",
                                                     name=f"av_{hp}_{q0}_{hi}"))
                            for kc in range(NC):
                                # Both heads' scores land in one contiguous
                                # 2-bank PSUM tile so a single ACT does both
                                # exps (the ~220ns/instruction ACT overhead
                                # was making the scalar engine co-critical).
                                ss2 = psS.tile([128, 2, qg_max], f32, tag="ss")
                                for hi, h in enumerate(heads):
                                    lo = hi * 64
                                    for s0, ssz in _chunks(qsz, 512):
                                        nc.tensor.matmul(
                                            ss2[:, hi, s0:s0 + ssz],
                                            qkT[lo:lo + 64, 4 + hp, kc * 128:(kc + 1) * 128],
                                            qkT[lo:lo + 64, hp, q0 + s0:q0 + s0 + ssz],
                                            start=True, stop=True,
                                        )
                                pT2 = p_pT.tile([128, 2, qg_max], f16, tag="pT")
                                nc.scalar.activation(
                                    pT2[:, :, :qsz], ss2[:, :, :qsz], Exp,
                                    scale=0.125
                                )
                                for hi, h in enumerate(heads):
                                    for s0, ssz in _chunks(qsz, 512):
                                        nc.tensor.matmul(
                                            avs[hi][:, s0:s0 + ssz],
                                            vsb[:, kc, h, :],
                                            pT2[:, hi, s0:s0 + ssz],
                                            start=(kc == 0), stop=(kc == NC - 1),
                                        )
                            for hi, h in enumerate(heads):
                                av = avs[hi]
                                # 1/denom on DVE (fast approx, ~18 bits), then
                                # broadcast across 64 partitions via a DRAM
                                # round-trip DMA (keeps the PE out of it).
                                dcp = p_att.tile([1, qg_max], f32, tag="dcp")
                                nc.vector.tensor_copy(dcp[0:1, :qsz],
                                                      av[64:65, :qsz])
                                rec = p_att.tile([1, qg_max], f32, tag="rec")
                                nc.vector.reciprocal_approx_fast(
                                    rec[0:1, :qsz], dcp[0:1, :qsz])
                                r = (hp * len(qgroups) + qi) * 2 + hi
                                nc.sync.dma_start(nscr[r:r + 1, :qsz],
                                                  rec[0:1, :qsz])
                                bcs = p_att.tile([64, qg_max], f32, tag="bcs")
                                nc.sync.dma_start(
                                    bcs[:, :qsz],
                                    nscr[r:r + 1, :qsz].partition_broadcast(64))
                                lo = (h % 2) * 64
                                if has_bv:
                                    tmp = p_att.tile([64, qg_max], f16, tag="tmpb")
                                    nc.vector.tensor_mul(
                                        tmp[:, :qsz], av[0:64, :qsz], bcs[:, :qsz])
                                    nc.gpsimd.tensor_scalar_add(
                                        attnT[lo:lo + 64, h // 2, q0:q0 + qsz],
                                        tmp[:, :qsz],
                                        bvTs[lo:lo + 64, h // 2:h // 2 + 1])
                                else:
                                    nc.vector.tensor_mul(
                                        attnT[lo:lo + 64, h // 2, q0:q0 + qsz],
                                        av[0:64, :qsz],
                                        bcs[:, :qsz],
                                    )

                # phase C: output projection partial, transposed out
                # chunk-outer so the first 2/3 of the projection (which only
                # needs earlier qgroups' attnT) runs while the last qgroup's
                # normalize chain drains.
                with tc.tile_pool(name="psC", bufs=4, space="PSUM") as psC:
                    for ci, (n0, nsz) in enumerate(nq512):
                        for m in range(8):
                            ps = psC.tile([128, 512], f32, tag="psC")
                            for j in range(HPC // 2):
                                nc.tensor.matmul(
                                    ps[:, :nsz],
                                    wps[:, j, m * 128:(m + 1) * 128],
                                    attnT[:, j, n0:n0 + nsz],
                                    start=(j == 0), stop=(j == HPC // 2 - 1),
                                )
                            st = p_out.tile([128, 512], f16, tag="st")
                            nc.vector.tensor_copy(st[:, :nsz], ps[:, :nsz])
                            nc.sync.dma_start(
                                outT[m * 128:(m + 1) * 128, n0:n0 + nsz], st[:, :nsz]
                            )

    nc.finalize()
    return nc


def _get_nc(Cp, nq, has_bqk, has_bv):
    key = (Cp, nq, has_bqk, has_bv)
    if key not in _NC_CACHE:
        _NC_CACHE[key] = _build_nc(Cp, nq, has_bqk, has_bv)
    return _NC_CACHE[key]


def kernel(x, mask_ind, Wqkv, bqkv, Wproj, bproj, **_unused):
    from concourse.bass_utils import run_bass_kernel_spmd

    x = np.asarray(x, dtype=np.float32)
    mask_ind = np.asarray(mask_ind)
    Wqkv = np.asarray(Wqkv, dtype=np.float32)
    bqkv = np.asarray(bqkv, dtype=np.float32)
    Wproj = np.asarray(Wproj, dtype=np.float32)
    bproj = np.asarray(bproj, dtype=np.float32)

    # kept-token sets per batch (matches reference _keep_mask semantics)
    idx = []
    for b in range(B):
        mi = mask_ind[b]
        mi = mi[mi >= 0]
        mi = np.clip(mi, 0, C - 1)
        idx.append(np.unique(mi).astype(np.int64))
    nmax = max(1, max(len(u) for u in idx))
    Cp = max(128, ((nmax + 127) // 128) * 128)
    NC = Cp // 128

    has_bqk = bool(np.any(bqkv[:2 * D] != 0.0))
    has_bv = bool(np.any(bqkv[2 * D:] != 0.0))
    nc = _get_nc(Cp, nmax, has_bqk, has_bv)

    in_maps = []
    for core in range(N_CORES):
        b, g = core // 2, core % 2
        u = idx[b]
        n = len(u)
        xk = np.zeros((Cp, D), dtype=np.float16)
        xk[:n] = x[b, u]
        keep = np.zeros(Cp, dtype=np.float32)
        keep[:n] = 1.0
        qs, ks, vs = g * FQ, D + g * FQ, 2 * D + g * FQ
        wqk = np.concatenate(
            [Wqkv[:, qs:qs + FQ], Wqkv[:, ks:ks + FQ]], axis=1
        ).astype(np.float16)
        im = {
            "xT": np.ascontiguousarray(xk.T),
            "wqk": np.ascontiguousarray(wqk),
            "wv": np.ascontiguousarray(Wqkv[:, vs:vs + FQ]).astype(np.float16),
            "wp": np.ascontiguousarray(Wproj[g * FQ:(g + 1) * FQ, :]).astype(np.float16),
            "keep": np.ascontiguousarray(keep.reshape(NC, 128).T),
            "keeph": np.ascontiguousarray(keep.reshape(NC, 128).T).astype(np.float16),
        }
        if has_bqk:
            bqk = np.concatenate([bqkv[qs:qs + FQ], bqkv[ks:ks + FQ]])
            im["bqkT"] = np.ascontiguousarray(bqk.reshape(8, 128).T)
        if has_bv:
            im["bvT"] = np.ascontiguousarray(
                bqkv[vs:vs + FQ].reshape(4, 128).T)
        in_maps.append(im)

    global _last_in_maps
    _last_in_maps = in_maps

    # Very rare transient non-finite outputs have been observed on a hot
    # device (soft error); one retry makes that a non-event.
    for _attempt in range(3):
        res = run_bass_kernel_spmd(nc, in_maps, core_ids=list(range(N_CORES)))
        parts = [np.asarray(r["outT"]).astype(np.float32) for r in res.results]
        if all(np.isfinite(p).all() for p in parts):
            break

    out = np.broadcast_to(bproj, (B, C, D)).copy()
    for b in range(B):
        u = idx[b]
        n = len(u)
        comb = parts[2 * b] + parts[2 * b + 1]
        out[b, u] += comb.T[:n]
    return out
